# revision 49
# baseline (speedup 1.0000x reference)
"""Causal self-attention (B=2, S=2048, E=1024, H=16) on 8 trn2 cores.

Sharding: batch x head -- core c handles batch c//4 and the 4 heads
starting at (c%4)*4. Each core runs QKV projection for its heads,
causal attention, and its slice of the output projection (row-split
c_proj); the host sums the 4 partial projections per batch.

v3: latency-shaped single pipeline.
- Attention is software-pipelined at score-pair granularity: the AV
  matmul of pair i is emitted one pair late, with qk/v/proj "filler"
  matmuls woven between, so the PE's in-order queue never stalls on
  the scalar-engine exp round-trip.
- q-tiles run 0->1->2->3 so exp work starts as soon as the first QK
  projections land and all 16 output-projection tiles stream out
  mid-kernel (no output burst at the tail).
- Input streams ec-major; the QK projection for q-tiles 0,1 uses all
  8 PSUM banks and consumes each xT chunk as it arrives.
- kT is stored unpadded ([128, pair, S] like qT); scores contract 64
  partitions via PE quadrant tiling instead of zero-padding.
- Softmax normalization: rowsums ride the AV matmul as a ones-column;
  per q-tile one gather DMA + one Ln + one Exp computes all 4 heads'
  reciprocals; a tiny stationary-indicator matmul broadcasts them.
"""

import os
import sys

import numpy as np

_DIR = os.path.dirname(os.path.abspath(__file__))
for _p in (_DIR,):
    if _p not in sys.path:
        sys.path.insert(0, _p)

import concourse.bass as bass
import concourse.mybir as mybir
from concourse import tile
from concourse.vector_clock import ScopedClock, VectorClock

F32 = mybir.dt.float32
F32R = mybir.dt.float32r
F16 = mybir.dt.float16
U16 = mybir.dt.uint16

B, S, E, H, D = 2, 2048, 1024, 16, 64
HPC = 4          # heads per core
N_CORES = 8
QT = 512         # q tile (moving dim)
KC = 128         # k chunk (contraction tile)
EC = E // 128    # 8 contraction chunks over the embedding dim
NQ = S // QT     # 4 q tiles
NST = S // 128   # 16 s tiles of 128
NSLAB = 8        # yTun ring depth (q-tile x head slabs)


class SplitDrainTileContext(tile.TileContext):
    """Kernel-tail drain with its sem waits split one per instruction.

    The walrus build here rejects instructions carrying more sync waits
    than their ISA struct encodes; TileContext hangs one wait per live
    proc on a single Drain. Sequential single-wait drains on the sync
    engine give the same guarantee.
    """

    def _drain_and_barrier(self, tick_clock, wait_clock):
        gc = list(tick_clock.global_clock)
        n = len(gc)
        for i, t in enumerate(gc):
            if t:
                vc = VectorClock([t if j == i else 0 for j in range(n)])
                inst = self.nc.sync.drain()
                wait_clock.add_sem_waits(inst.ins, ScopedClock({None: vc}))
        self.nc.all_engine_barrier()
        assert self.sems is not None
        popped = self.nc._tile_sem_poison_stack.pop()
        assert popped is self._sem_poison
        self.nc.clear_and_free_semaphores(list(self.sems.allocated().values()))
        self.nc.all_engine_barrier()


# ---------------------------------------------------------------- BIR fix

_CAPS = {"EventSemaphore": 2}
_DEFAULT_CAP = 1
_counter = [0]


def _split_bir_waits(bir):
    """Move excess sync waits onto EventSemaphores inserted just before
    the overloaded instruction (same engine => same program order)."""
    n = 0
    for fn in bir.get("functions", []):
        for bb in fn.get("blocks", []):
            out = []
            for inst in bb.get("instructions", []):
                si = inst.get("sync_info")
                waits = si.get("on_wait") if si else None
                cap = _CAPS.get(inst.get("opcode"), _DEFAULT_CAP)
                if waits and len(waits) > cap:
                    excess, keep = waits[:-cap], waits[-cap:]
                    for i in range(0, len(excess), 2):
                        _counter[0] += 1
                        out.append({
                            "debug": inst.get("debug", 0),
                            "engine": inst["engine"],
                            "ins": [], "outs": [],
                            "name": f"antsplitw-{_counter[0]}",
                            "opcode": "EventSemaphore",
                            "sync_info": {"on_update": [],
                                          "on_wait": excess[i:i + 2]},
                        })
                        n += 1
                    si["on_wait"] = keep
                out.append(inst)
            bb["instructions"] = out
    return n


def _install_bir_fix():
    import json
    import concourse.bass2jax as bass2jax
    from concourse.bass_utils import compile_bir_kernel as orig
    if getattr(bass2jax.compile_bir_kernel, "_ant_split", False):
        return

    def wrapped(ant_bir_str, *args, **kwargs):
        bir = json.loads(ant_bir_str)
        if _split_bir_waits(bir):
            ant_bir_str = json.dumps(bir).encode()
        return orig(ant_bir_str, *args, **kwargs)

    wrapped._ant_split = True
    bass2jax.compile_bir_kernel = wrapped


# ---------------------------------------------------------------- device

def build():
    nc = bass.Bass("TRN2", target_bir_lowering=False, debug=False)
    # all inputs host-repacked partition-major: [128, chunk, cols], so every
    # DMA line is a contiguous run >= 2 KB
    xT_d = nc.dram_tensor("xT", [128, EC, S], F16, kind="ExternalInput").ap()
    wqk_d = nc.dram_tensor("wqk", [128, EC, 512], F16, kind="ExternalInput").ap()
    wv_d = nc.dram_tensor("wv", [128, EC, 256], F16, kind="ExternalInput").ap()
    wp_d = nc.dram_tensor("wproj", [128, 2, E], F16, kind="ExternalInput").ap()
    y_d = nc.dram_tensor("y", [S, E], F16, kind="ExternalOutput").ap()

    with SplitDrainTileContext(nc) as tc:
        with (
            tc.tile_pool(name="persist", bufs=1) as persist,
            tc.tile_pool(name="ptp", bufs=4) as ptp,
            tc.tile_pool(name="nrm", bufs=2) as nrm,
            tc.tile_pool(name="pout", bufs=4) as pout,
            tc.tile_pool(name="ps1", bufs=2, space="PSUM") as ps1,
            tc.tile_pool(name="pss", bufs=2, space="PSUM") as pss,
            tc.tile_pool(name="pav", bufs=2, space="PSUM") as pav,
        ):
            xT_sb = persist.tile([128, EC, S], F16)
            wqk_sb = persist.tile([128, EC, 512], F16)
            wv_sb = persist.tile([128, EC, 256], F16)
            wp_sb = persist.tile([128, 2, E], F16)
            qT = persist.tile([128, 2, S], F16)    # heads 01 | 23 stacked
            kTpad = persist.tile([128, HPC, S], F16)  # per head, half zero
            vaug = persist.tile([128, NST, HPC, D + 1], F16)
            yT = persist.tile([128, 2, S], F16)    # normalized, proj lhsT
            yTun = persist.tile([65, NSLAB, QT], F32)  # unnormalized + rowsum
            # recip-broadcast operands: ind33 row0 -> out cols 0:64, row32 ->
            # cols 64:128; rows 1..31 stay zero so stale rtp rows can't leak
            ind33 = persist.tile([33, 128], F32R)
            rtp = persist.tile([33, 2, QT], F32R)  # ping-pong recip rows

            # ---- input DMA kickoff, ec-major consumption order ----
            # round-robin across three queues so arrival tracks issue order
            qrot = (nc.sync, nc.scalar, nc.gpsimd)
            _qi = [0]

            def in_dma(dst, src):
                qrot[_qi[0] % 3].dma_start(dst, src)
                _qi[0] += 1

            # phase A (QK proj, q-tiles 0,1) reads only token cols 0:1024,
            # so stream all first-halves (+ wqk) before second-halves.
            def xa(ec):
                in_dma(xT_sb[:, ec, 0:1024], xT_d[:, ec, 0:1024])

            def xb(ec):
                in_dma(xT_sb[:, ec, 1024:2048], xT_d[:, ec, 1024:2048])

            # tiny first slivers so the first matmuls fire ASAP
            in_dma(wqk_sb[:, 0:1, :], wqk_d[:, 0:1, :])
            in_dma(xT_sb[:, 0, 0:512], xT_d[:, 0, 0:512])
            in_dma(xT_sb[:, 0, 512:1024], xT_d[:, 0, 512:1024])
            in_dma(wqk_sb[:, 1:2, :], wqk_d[:, 1:2, :])
            xa(1)
            xa(2)
            in_dma(wqk_sb[:, 2:4, :], wqk_d[:, 2:4, :])
            xa(3)
            xa(4)
            in_dma(wqk_sb[:, 4:6, :], wqk_d[:, 4:6, :])
            xa(5)
            xa(6)
            in_dma(wqk_sb[:, 6:8, :], wqk_d[:, 6:8, :])
            xa(7)
            in_dma(wv_sb[:], wv_d[:])
            for ec in range(EC):
                xb(ec)
            in_dma(wp_sb[:], wp_d[:])

            nc.vector.memset(vaug[:, :, :, D:D + 1].bitcast(U16), 15360)
            nc.vector.memset(ind33[:].bitcast(F32), 0.0)
            nc.vector.memset(rtp[:].bitcast(F32), 0.0)
            nc.vector.memset(ind33[0:1, 0:64].bitcast(F32), 1.0)
            nc.vector.memset(ind33[32:33, 64:128].bitcast(F32), 1.0)
            for h in range(HPC):
                dead = slice(64, 128) if h % 2 == 0 else slice(0, 64)
                nc.vector.memset(kTpad[dead, h, :].bitcast(U16), 0)

            # ---- QK projection for q-tiles 0,1: all 8 PSUM banks ----
            def qk_copy_out(rt, q4, ps_, eng):
                sslc = slice(q4 * QT, (q4 + 1) * QT)
                if rt < 2:
                    if eng is nc.scalar:
                        eng.copy(qT[:, rt, sslc], ps_[0:128, :])
                    else:
                        eng.tensor_copy(out=qT[:, rt, sslc],
                                        in_=ps_[0:128, :])
                else:
                    h2 = 2 * (rt - 2)
                    if eng is nc.scalar:
                        eng.copy(kTpad[0:64, h2, sslc], ps_[0:64, :])
                    else:
                        eng.tensor_copy(out=kTpad[0:64, h2, sslc],
                                        in_=ps_[0:64, :])
                    nc.vector.tensor_copy(out=kTpad[64:128, h2 + 1, sslc],
                                          in_=ps_[64:128, :])

            def qk_ec_major():
                # acc->pool map and copy order tuned so the PSUM bufs the
                # attention pipeline needs first are freed first:
                # ps1 (v-units) frees at copy #3, pss-A (first ss pair) at
                # copy #4, pss-B at #5.
                ssA = pss.tile([128, 2, QT], F32, tag="ss", name="qkA")
                ssB = pss.tile([128, 2, QT], F32, tag="ss", name="qkB")
                m0 = ps1.tile([128, QT], F32, tag="m", name="qkm0")
                m1 = ps1.tile([128, QT], F32, tag="m", name="qkm1")
                a0 = pav.tile([128, QT], F32, tag="av", name="qka0")
                a1 = pav.tile([128, QT], F32, tag="av", name="qka1")
                accs = {(2, 0): ssA[:, 0, :], (3, 0): ssA[:, 1, :],
                        (0, 0): ssB[:, 0, :], (1, 0): ssB[:, 1, :],
                        (2, 1): m0[:], (3, 1): m1[:],
                        (0, 1): a0[:], (1, 1): a1[:]}
                for ec in range(EC):
                    for rt in range(4):
                        for q4 in range(2):
                            nc.tensor.matmul(
                                accs[(rt, q4)],
                                wqk_sb[:, ec, rt * 128:(rt + 1) * 128],
                                xT_sb[:, ec, q4 * QT:(q4 + 1) * QT],
                                start=(ec == 0), stop=(ec == EC - 1))
                cord = [(2, 0), (0, 0), (2, 1), (3, 1),
                        (3, 0), (1, 0), (0, 1), (1, 1)]
                for i, (rt, q4) in enumerate(cord):
                    eng = nc.vector if i % 2 == 0 else nc.scalar
                    qk_copy_out(rt, q4, accs[(rt, q4)], eng)

            # ---- filler units (one closure = a short PE burst) ----
            def qk23_unit(rt, q4):
                def emit():
                    acc = ps1.tile([128, QT], F32, tag="m",
                                   name=f"qk{rt}{q4}")
                    for ec in range(EC):
                        nc.tensor.matmul(
                            acc[:], wqk_sb[:, ec, rt * 128:(rt + 1) * 128],
                            xT_sb[:, ec, q4 * QT:(q4 + 1) * QT],
                            start=(ec == 0), stop=(ec == EC - 1))
                    qk_copy_out(rt, q4, acc, nc.vector)
                return emit

            def v_unit(st2):
                def emit():
                    pv = ps1.tile([128, 256], F32, tag="m", name=f"pv{st2}")
                    for ec in range(EC):
                        nc.tensor.matmul(
                            pv[:], xT_sb[:, ec, st2 * 128:(st2 + 1) * 128],
                            wv_sb[:, ec, :],
                            start=(ec == 0), stop=(ec == EC - 1))
                    nc.vector.tensor_copy(
                        out=vaug[:, st2, :, 0:D],
                        in_=pv[:, :].rearrange("p (h d) -> p h d", h=HPC))
                return emit

            _oq = [0]

            def proj_unit(qt, cengs=None, dengs=None):
                def emit():
                    halves = [ps1.tile([128, QT], F32, tag="m",
                                       name=f"pp{qt}{eo}")[:]
                              for eo in range(2)]
                    for eo in range(2):
                        for ci in range(2):
                            nc.tensor.matmul(
                                halves[eo],
                                yT[:, ci, qt * 128:(qt + 1) * 128],
                                wp_sb[:, ci, eo * 512:(eo + 1) * 512],
                                start=(ci == 0), stop=(ci == 1))
                    po_t = pout.tile([128, 2 * QT], F16, tag="po",
                                     name=f"po{qt}")
                    for eo in range(2):
                        oslc = po_t[:, eo * QT:(eo + 1) * QT]
                        eng = (cengs[eo] if cengs is not None
                               else nc.vector)
                        if eng is nc.scalar:
                            eng.copy(oslc, halves[eo])
                        else:
                            eng.tensor_copy(out=oslc, in_=halves[eo])
                        if dengs is not None:
                            # tail: drain each half as soon as it's copied,
                            # on its own queue
                            dengs[eo].dma_start(
                                y_d[qt * 128:(qt + 1) * 128,
                                    eo * QT:(eo + 1) * QT],
                                po_t[:, eo * QT:(eo + 1) * QT])
                    _oq[0] += 1
                    if dengs is None:
                        nc.gpsimd.dma_start(
                            y_d[qt * 128:(qt + 1) * 128, :], po_t[:])
                return emit

            # ---- attention pair pipeline ----
            _pend = [None]      # AV closure of the previous pair

            def s_pair(qj, h, pr):
                """Scores pair matmuls + exp + diagonal mask; returns the
                AV-emitter closure (run one pair later)."""
                hp = h // 2
                nkc = (qj + 1) * QT // KC
                ss = pss.tile([128, 2, QT], F32, tag="ss",
                              name=f"ss{qj}{h}{pr}")
                offs = [max(0, (2 * pr + j) * KC - qj * QT)
                        for j in range(2)]
                for j in range(2):
                    kc = 2 * pr + j
                    o = offs[j]
                    nc.tensor.matmul(
                        ss[:, j, o:QT],
                        kTpad[:, h, kc * KC:(kc + 1) * KC],
                        qT[:, hp, qj * QT + o:(qj + 1) * QT],
                        start=True, stop=True)
                pt = ptp.tile([128, 2, QT], F16, tag="pt",
                              name=f"pt{qj}{h}{pr}")
                if sum(offs) < 352:
                    # one exp for the pair; any dead-region garbage is
                    # never read (AV slices [o:])
                    nc.scalar.activation(
                        pt[:], ss[:],
                        mybir.ActivationFunctionType.Exp, scale=0.125)
                else:
                    # deep-diagonal pair: exp only live columns
                    for j, o in enumerate(offs):
                        nc.scalar.activation(
                            pt[:, j, o:QT], ss[:, j, o:QT],
                            mybir.ActivationFunctionType.Exp, scale=0.125)
                for j in range(2):
                    kc = 2 * pr + j
                    if kc * KC >= qj * QT:
                        # mask only the 128-wide diagonal band
                        o = offs[j]
                        w = min(KC, QT - o)
                        nc.gpsimd.affine_select(
                            out=pt[:, j, o:o + w],
                            in_=pt[:, j, o:o + w],
                            compare_op=mybir.AluOpType.is_ge,
                            fill=0.0, base=qj * QT + o - kc * KC,
                            pattern=[[1, w]],
                            channel_multiplier=-1)

                def av_emit(av):
                    for j in range(2):
                        kc = 2 * pr + j
                        o = offs[j]
                        nc.tensor.matmul(av[:, o:QT],
                                         vaug[:, kc, h, :],
                                         pt[:, j, o:QT],
                                         start=(kc == 0),
                                         stop=(kc == nkc - 1))
                    if pr == nkc // 2 - 1:  # last pair of this head
                        slab = (qj * HPC + h) % NSLAB
                        nc.vector.tensor_copy(out=yTun[:, slab, :],
                                              in_=av[:])
                return av_emit

            _avt = {}           # (qj, h) -> live av accumulator tile

            def attn_pair(qj, h, pr):
                av_emit = s_pair(qj, h, pr)
                if _pend[0] is not None:
                    _pend[0]()
                if pr == 0:
                    _avt[(qj, h)] = pav.tile([65, QT], F32, tag="av",
                                             name=f"av{qj}{h}")
                av = _avt[(qj, h)]
                _pend[0] = lambda: av_emit(av[:])

            def flush_av():
                if _pend[0] is not None:
                    _pend[0]()
                    _pend[0] = None

            # ---- normalization, split into two units so the bc matmul
            # never reaches the in-order PE queue before its reciprocal
            # (DMA -> Ln -> Exp -> DMA) is ready: a sub-us PE stall here
            # costs a multi-us HAM half-clock window.
            def recip_unit(qj, p):
                """Reciprocal of the rowsums for one head pair; rtp slot =
                pair index (the previous q-tile's reader is long done)."""
                def emit():
                    s0 = (qj * HPC + 2 * p) % NSLAB
                    rs2 = nrm.tile([2, QT], F32, tag="rs",
                                   name=f"rs{qj}{p}")
                    nc.sync.dma_start(rs2[:, :], yTun[64:65, s0:s0 + 2, :])
                    lg2 = nrm.tile([2, QT], F32, tag="lg",
                                   name=f"lg{qj}{p}")
                    nc.scalar.activation(lg2[:], rs2[:],
                                         mybir.ActivationFunctionType.Ln)
                    rt2 = nrm.tile([2, QT], F32R, tag="rt",
                                   name=f"rt{qj}{p}")
                    nc.scalar.activation(rt2[:], lg2[:],
                                         mybir.ActivationFunctionType.Exp,
                                         scale=-1.0)
                    for i in range(2):
                        nc.gpsimd.dma_start(rtp[32 * i:32 * i + 1, p, :],
                                            rt2[i:i + 1, :])
                return emit

            def recip4_unit(qj):
                """Batched reciprocal for all 4 heads of a q-tile: one
                gather, one Ln, one Exp (free size unchanged at 512)."""
                def emit():
                    s0 = (qj * HPC) % NSLAB
                    rs4 = nrm.tile([4, QT], F32, tag="rs",
                                   name=f"rs4{qj}")
                    nc.sync.dma_start(rs4[:, :], yTun[64:65, s0:s0 + 4, :])
                    lg4 = nrm.tile([4, QT], F32, tag="lg",
                                   name=f"lg4{qj}")
                    nc.scalar.activation(lg4[:], rs4[:],
                                         mybir.ActivationFunctionType.Ln)
                    rt4 = nrm.tile([4, QT], F32R, tag="rt",
                                   name=f"rt4{qj}")
                    nc.scalar.activation(rt4[:], lg4[:],
                                         mybir.ActivationFunctionType.Exp,
                                         scale=-1.0)
                    for h in range(HPC):
                        nc.gpsimd.dma_start(
                            rtp[32 * (h % 2):32 * (h % 2) + 1, h // 2, :],
                            rt4[h:h + 1, :])
                return emit

            def bcmul_unit(qj, p):
                def emit():
                    qslc = slice(qj * QT, (qj + 1) * QT)
                    bc = ps1.tile([128, QT], F32, tag="m",
                                  name=f"bc{qj}{p}")
                    nc.tensor.matmul(bc[:], ind33[:, :], rtp[:, p, :],
                                     start=True, stop=True)
                    for i in range(2):
                        h = 2 * p + i
                        slab = (qj * HPC + h) % NSLAB
                        po = 64 * i
                        with nc.allow_low_precision(reason="proj lhsT"):
                            nc.vector.tensor_tensor(
                                out=yT[po:po + 64, p, qslc],
                                in0=yTun[0:64, slab, :],
                                in1=bc[po:po + 64, :],
                                op=mybir.AluOpType.mult)
                return emit

            # ---- merged pipeline ----
            qk_ec_major()
            v_unit(0)()
            v_unit(1)()

            # filler schedule per q-tile: {slot: [units]} -- slots are
            # attention-pair indices; units placed just before their
            # consumers need them so the PE stream never runs dry (HAM
            # keeps the PE clock at 2.4 GHz only while it stays busy).
            fills = {
                0: {0: [v_unit(2)], 1: [v_unit(3)], 2: [v_unit(4)],
                    3: [v_unit(5)], 4: [v_unit(6)], 5: [v_unit(7)],
                    6: [qk23_unit(2, 2)], 7: [qk23_unit(3, 2)]},
                1: {0: [recip4_unit(0)], 1: [v_unit(8)],
                    3: [bcmul_unit(0, 0)], 4: [bcmul_unit(0, 1)],
                    5: [v_unit(9)], 7: [v_unit(10)],
                    9: [v_unit(11)], 11: [qk23_unit(0, 2)],
                    13: [qk23_unit(1, 2)]},
                2: {0: [recip4_unit(1)], 1: [proj_unit(0)],
                    2: [qk23_unit(0, 3)], 3: [bcmul_unit(1, 0)],
                    4: [bcmul_unit(1, 1), proj_unit(1)],
                    5: [qk23_unit(1, 3)], 7: [proj_unit(2)],
                    9: [proj_unit(3)], 11: [proj_unit(4)],
                    13: [proj_unit(5)], 15: [proj_unit(6)],
                    17: [proj_unit(7)],
                    18: [v_unit(12)], 19: [v_unit(13)],
                    20: [v_unit(14)], 21: [v_unit(15)]},
                3: {0: [recip4_unit(2)], 1: [qk23_unit(2, 3)],
                    2: [qk23_unit(3, 3)], 3: [bcmul_unit(2, 0)],
                    4: [bcmul_unit(2, 1)],
                    9: [proj_unit(8)],
                    13: [proj_unit(9)], 19: [recip_unit(3, 0)],
                    22: [bcmul_unit(3, 0)],
                    26: [proj_unit(10, dengs=(nc.sync, nc.scalar))]},
            }

            for qj in range(4):
                npairs = (qj + 1) * QT // KC // 2
                idx = 0
                for h in range(HPC):
                    for pr in range(npairs):
                        attn_pair(qj, h, pr)
                        for u in fills[qj].get(idx, ()):
                            u()
                        idx += 1
            # tail: the last recip chain runs under proj 10/11 matmuls so
            # the PE never idles while it resolves
            flush_av()
            recip_unit(3, 1)()
            proj_unit(11, dengs=(nc.scalar, nc.sync))()
            bcmul_unit(3, 1)()
            proj_unit(12, cengs=(nc.scalar, nc.vector),
                      dengs=(nc.sync, nc.scalar))()
            proj_unit(13, cengs=(nc.vector, nc.scalar),
                      dengs=(nc.scalar, nc.sync))()
            proj_unit(14, cengs=(nc.scalar, nc.vector),
                      dengs=(nc.sync, nc.scalar))()
            proj_unit(15, cengs=(nc.vector, nc.scalar),
                      dengs=(nc.scalar, nc.sync))()
    return nc


# ---------------------------------------------------------------- host

_NC_CACHE = []


def _get_nc():
    if not _NC_CACHE:
        _install_bir_fix()
        _NC_CACHE.append(build())
    return _NC_CACHE[0]


def make_in_maps(x, w_attn, w_proj):
    in_maps = []
    for c in range(N_CORES):
        b, h0 = c // 4, (c % 4) * HPC
        wq = w_attn[:, h0 * D:(h0 + HPC) * D]
        wk = w_attn[:, E + h0 * D:E + (h0 + HPC) * D]
        wv = w_attn[:, 2 * E + h0 * D:2 * E + (h0 + HPC) * D]
        def pmaj(a, dt):
            # [n*128, c] row-chunked -> partition-major [128, n, c]
            n = a.shape[0] // 128
            return np.ascontiguousarray(
                a.reshape(n, 128, -1).transpose(1, 0, 2).astype(dt))

        in_maps.append({
            "xT": pmaj(x[b].T, np.float16),
            "wqk": pmaj(np.concatenate([wq, wk], axis=1), np.float16),
            "wv": pmaj(wv, np.float16),
            "wproj": pmaj(w_proj[h0 * D:(h0 + HPC) * D, :], np.float16),
        })
    return in_maps


def run(x, w_attn, w_proj, trace=False, tmpdir=None):
    from concourse.bass_utils import run_bass_kernel_spmd
    nc = _get_nc()
    res = run_bass_kernel_spmd(nc, make_in_maps(x, w_attn, w_proj),
                               list(range(N_CORES)), trace=trace, tmpdir=tmpdir)
    y = np.zeros((B, S, E), np.float32)
    for c in range(N_CORES):
        y[c // 4] += res.results[c]["y"].astype(np.float32)
    return y, res


def kernel(x, w_attn, w_proj):
    y, _ = run(np.asarray(x, np.float32), np.asarray(w_attn, np.float32),
               np.asarray(w_proj, np.float32))
    return y


# revision 51
# speedup vs baseline: 1.0225x; 1.0225x over previous
"""Causal self-attention (B=2, S=2048, E=1024, H=16) on 8 trn2 cores.

Sharding: batch x head -- core c handles batch c//4 and the 4 heads
starting at (c%4)*4. Each core runs QKV projection for its heads,
causal attention, and its slice of the output projection (row-split
c_proj); the host sums the 4 partial projections per batch.

v3: latency-shaped single pipeline.
- Attention is software-pipelined at score-pair granularity: the AV
  matmul of pair i is emitted one pair late, with qk/v/proj "filler"
  matmuls woven between, so the PE's in-order queue never stalls on
  the scalar-engine exp round-trip.
- q-tiles run 0->1->2->3 so exp work starts as soon as the first QK
  projections land and all 16 output-projection tiles stream out
  mid-kernel (no output burst at the tail).
- Input streams ec-major; the QK projection for q-tiles 0,1 uses all
  8 PSUM banks and consumes each xT chunk as it arrives.
- kT is stored unpadded ([128, pair, S] like qT); scores contract 64
  partitions via PE quadrant tiling instead of zero-padding.
- Softmax normalization: rowsums ride the AV matmul as a ones-column;
  per q-tile one gather DMA + one Ln + one Exp computes all 4 heads'
  reciprocals; a tiny stationary-indicator matmul broadcasts them.
"""

import os
import sys

import numpy as np

_DIR = os.path.dirname(os.path.abspath(__file__))
for _p in (_DIR,):
    if _p not in sys.path:
        sys.path.insert(0, _p)

import concourse.bass as bass
import concourse.mybir as mybir
from concourse import tile
from concourse.vector_clock import ScopedClock, VectorClock

F32 = mybir.dt.float32
F32R = mybir.dt.float32r
F16 = mybir.dt.float16
U16 = mybir.dt.uint16

B, S, E, H, D = 2, 2048, 1024, 16, 64
HPC = 4          # heads per core
N_CORES = 8
QT = 512         # q tile (moving dim)
KC = 128         # k chunk (contraction tile)
EC = E // 128    # 8 contraction chunks over the embedding dim
NQ = S // QT     # 4 q tiles
NST = S // 128   # 16 s tiles of 128
NSLAB = 8        # yTun ring depth (q-tile x head slabs)


class SplitDrainTileContext(tile.TileContext):
    """Kernel-tail drain with its sem waits split one per instruction.

    The walrus build here rejects instructions carrying more sync waits
    than their ISA struct encodes; TileContext hangs one wait per live
    proc on a single Drain. Sequential single-wait drains on the sync
    engine give the same guarantee.
    """

    def _drain_and_barrier(self, tick_clock, wait_clock):
        gc = list(tick_clock.global_clock)
        n = len(gc)
        for i, t in enumerate(gc):
            if t:
                vc = VectorClock([t if j == i else 0 for j in range(n)])
                inst = self.nc.sync.drain()
                wait_clock.add_sem_waits(inst.ins, ScopedClock({None: vc}))
        self.nc.all_engine_barrier()
        assert self.sems is not None
        popped = self.nc._tile_sem_poison_stack.pop()
        assert popped is self._sem_poison
        self.nc.clear_and_free_semaphores(list(self.sems.allocated().values()))
        self.nc.all_engine_barrier()


# ---------------------------------------------------------------- BIR fix

_CAPS = {"EventSemaphore": 2}
_DEFAULT_CAP = 1
_counter = [0]


def _split_bir_waits(bir):
    """Move excess sync waits onto EventSemaphores inserted just before
    the overloaded instruction (same engine => same program order)."""
    n = 0
    for fn in bir.get("functions", []):
        for bb in fn.get("blocks", []):
            out = []
            for inst in bb.get("instructions", []):
                si = inst.get("sync_info")
                waits = si.get("on_wait") if si else None
                cap = _CAPS.get(inst.get("opcode"), _DEFAULT_CAP)
                if waits and len(waits) > cap:
                    excess, keep = waits[:-cap], waits[-cap:]
                    for i in range(0, len(excess), 2):
                        _counter[0] += 1
                        out.append({
                            "debug": inst.get("debug", 0),
                            "engine": inst["engine"],
                            "ins": [], "outs": [],
                            "name": f"antsplitw-{_counter[0]}",
                            "opcode": "EventSemaphore",
                            "sync_info": {"on_update": [],
                                          "on_wait": excess[i:i + 2]},
                        })
                        n += 1
                    si["on_wait"] = keep
                out.append(inst)
            bb["instructions"] = out
    return n


def _install_bir_fix():
    import json
    import concourse.bass2jax as bass2jax
    from concourse.bass_utils import compile_bir_kernel as orig
    if getattr(bass2jax.compile_bir_kernel, "_ant_split", False):
        return

    def wrapped(ant_bir_str, *args, **kwargs):
        bir = json.loads(ant_bir_str)
        if _split_bir_waits(bir):
            ant_bir_str = json.dumps(bir).encode()
        return orig(ant_bir_str, *args, **kwargs)

    wrapped._ant_split = True
    bass2jax.compile_bir_kernel = wrapped


# ---------------------------------------------------------------- device

def build():
    nc = bass.Bass("TRN2", target_bir_lowering=False, debug=False)
    # all inputs host-repacked partition-major: [128, chunk, cols], so every
    # DMA line is a contiguous run >= 2 KB
    xT_d = nc.dram_tensor("xT", [128, EC, S], F16, kind="ExternalInput").ap()
    wqk_d = nc.dram_tensor("wqk", [128, EC, 512], F16, kind="ExternalInput").ap()
    wv_d = nc.dram_tensor("wv", [128, EC, 256], F16, kind="ExternalInput").ap()
    wp_d = nc.dram_tensor("wproj", [128, 2, E], F16, kind="ExternalInput").ap()
    y_d = nc.dram_tensor("y", [S, E], F16, kind="ExternalOutput").ap()

    with SplitDrainTileContext(nc) as tc:
        with (
            tc.tile_pool(name="persist", bufs=1) as persist,
            tc.tile_pool(name="ptp", bufs=6) as ptp,
            tc.tile_pool(name="nrm", bufs=3) as nrm,
            tc.tile_pool(name="pout", bufs=6) as pout,
            tc.tile_pool(name="ps1", bufs=2, space="PSUM") as ps1,
            tc.tile_pool(name="pss", bufs=2, space="PSUM") as pss,
            tc.tile_pool(name="pav", bufs=2, space="PSUM") as pav,
        ):
            xT_sb = persist.tile([128, EC, S], F16)
            wqk_sb = persist.tile([128, EC, 512], F16)
            wv_sb = persist.tile([128, EC, 256], F16)
            wp_sb = persist.tile([128, 2, E], F16)
            qT = persist.tile([128, 2, S], F16)    # heads 01 | 23 stacked
            kTpad = persist.tile([128, HPC, S], F16)  # per head, half zero
            vaug = persist.tile([128, NST, HPC, D + 1], F16)
            yT = persist.tile([128, 2, S], F16)    # normalized, proj lhsT
            yTun = persist.tile([65, NSLAB, QT], F32)  # unnormalized + rowsum
            # recip-broadcast operands: ind33 row0 -> out cols 0:64, row32 ->
            # cols 64:128; rows 1..31 stay zero so stale rtp rows can't leak
            ind33 = persist.tile([33, 128], F32R)
            rtp = persist.tile([33, 2, QT], F32R)  # ping-pong recip rows

            # ---- input DMA kickoff, ec-major consumption order ----
            # round-robin across three queues so arrival tracks issue order
            qrot = (nc.sync, nc.scalar, nc.gpsimd)
            _qi = [0]

            def in_dma(dst, src):
                qrot[_qi[0] % 3].dma_start(dst, src)
                _qi[0] += 1

            # phase A (QK proj, q-tiles 0,1) reads only token cols 0:1024,
            # so stream all first-halves (+ wqk) before second-halves.
            def xa(ec):
                in_dma(xT_sb[:, ec, 0:1024], xT_d[:, ec, 0:1024])

            def xb(ec):
                in_dma(xT_sb[:, ec, 1024:2048], xT_d[:, ec, 1024:2048])

            # tiny first slivers so the first matmuls fire ASAP
            in_dma(wqk_sb[:, 0:1, :], wqk_d[:, 0:1, :])
            in_dma(xT_sb[:, 0, 0:512], xT_d[:, 0, 0:512])
            in_dma(xT_sb[:, 0, 512:1024], xT_d[:, 0, 512:1024])
            in_dma(wqk_sb[:, 1:2, :], wqk_d[:, 1:2, :])
            xa(1)
            xa(2)
            in_dma(wqk_sb[:, 2:4, :], wqk_d[:, 2:4, :])
            xa(3)
            xa(4)
            in_dma(wqk_sb[:, 4:6, :], wqk_d[:, 4:6, :])
            xa(5)
            xa(6)
            in_dma(wqk_sb[:, 6:8, :], wqk_d[:, 6:8, :])
            xa(7)
            in_dma(wv_sb[:], wv_d[:])
            for ec in range(EC):
                xb(ec)
            in_dma(wp_sb[:], wp_d[:])

            nc.vector.memset(vaug[:, :, :, D:D + 1].bitcast(U16), 15360)
            nc.vector.memset(ind33[:].bitcast(F32), 0.0)
            nc.vector.memset(rtp[:].bitcast(F32), 0.0)
            nc.vector.memset(ind33[0:1, 0:64].bitcast(F32), 1.0)
            nc.vector.memset(ind33[32:33, 64:128].bitcast(F32), 1.0)
            for h in range(HPC):
                dead = slice(64, 128) if h % 2 == 0 else slice(0, 64)
                nc.vector.memset(kTpad[dead, h, :].bitcast(U16), 0)

            # ---- QK projection for q-tiles 0,1: all 8 PSUM banks ----
            def qk_copy_out(rt, q4, ps_, eng):
                sslc = slice(q4 * QT, (q4 + 1) * QT)
                if rt < 2:
                    if eng is nc.scalar:
                        eng.copy(qT[:, rt, sslc], ps_[0:128, :])
                    else:
                        eng.tensor_copy(out=qT[:, rt, sslc],
                                        in_=ps_[0:128, :])
                else:
                    h2 = 2 * (rt - 2)
                    if eng is nc.scalar:
                        eng.copy(kTpad[0:64, h2, sslc], ps_[0:64, :])
                    else:
                        eng.tensor_copy(out=kTpad[0:64, h2, sslc],
                                        in_=ps_[0:64, :])
                    nc.vector.tensor_copy(out=kTpad[64:128, h2 + 1, sslc],
                                          in_=ps_[64:128, :])

            def qk_ec_major():
                # acc->pool map and copy order tuned so the PSUM bufs the
                # attention pipeline needs first are freed first:
                # ps1 (v-units) frees at copy #3, pss-A (first ss pair) at
                # copy #4, pss-B at #5.
                ssA = pss.tile([128, 2, QT], F32, tag="ss", name="qkA")
                ssB = pss.tile([128, 2, QT], F32, tag="ss", name="qkB")
                m0 = ps1.tile([128, QT], F32, tag="m", name="qkm0")
                m1 = ps1.tile([128, QT], F32, tag="m", name="qkm1")
                a0 = pav.tile([128, QT], F32, tag="av", name="qka0")
                a1 = pav.tile([128, QT], F32, tag="av", name="qka1")
                accs = {(2, 0): ssA[:, 0, :], (3, 0): ssA[:, 1, :],
                        (0, 0): ssB[:, 0, :], (1, 0): ssB[:, 1, :],
                        (2, 1): m0[:], (3, 1): m1[:],
                        (0, 1): a0[:], (1, 1): a1[:]}
                for ec in range(EC):
                    for rt in range(4):
                        for q4 in range(2):
                            nc.tensor.matmul(
                                accs[(rt, q4)],
                                wqk_sb[:, ec, rt * 128:(rt + 1) * 128],
                                xT_sb[:, ec, q4 * QT:(q4 + 1) * QT],
                                start=(ec == 0), stop=(ec == EC - 1))
                cord = [(2, 0), (0, 0), (2, 1), (3, 1),
                        (3, 0), (1, 0), (0, 1), (1, 1)]
                for i, (rt, q4) in enumerate(cord):
                    eng = nc.vector if i % 2 == 0 else nc.scalar
                    qk_copy_out(rt, q4, accs[(rt, q4)], eng)

            # ---- filler units (one closure = a short PE burst) ----
            def qk23_unit(rt, q4):
                def emit():
                    acc = ps1.tile([128, QT], F32, tag="m",
                                   name=f"qk{rt}{q4}")
                    for ec in range(EC):
                        nc.tensor.matmul(
                            acc[:], wqk_sb[:, ec, rt * 128:(rt + 1) * 128],
                            xT_sb[:, ec, q4 * QT:(q4 + 1) * QT],
                            start=(ec == 0), stop=(ec == EC - 1))
                    qk_copy_out(rt, q4, acc, nc.vector)
                return emit

            def v_unit(st2):
                def emit():
                    pv = ps1.tile([128, 256], F32, tag="m", name=f"pv{st2}")
                    for ec in range(EC):
                        nc.tensor.matmul(
                            pv[:], xT_sb[:, ec, st2 * 128:(st2 + 1) * 128],
                            wv_sb[:, ec, :],
                            start=(ec == 0), stop=(ec == EC - 1))
                    nc.vector.tensor_copy(
                        out=vaug[:, st2, :, 0:D],
                        in_=pv[:, :].rearrange("p (h d) -> p h d", h=HPC))
                return emit

            _oq = [0]

            def proj_unit(qt, cengs=None, dengs=None):
                def emit():
                    halves = [ps1.tile([128, QT], F32, tag="m",
                                       name=f"pp{qt}{eo}")[:]
                              for eo in range(2)]
                    for eo in range(2):
                        for ci in range(2):
                            nc.tensor.matmul(
                                halves[eo],
                                yT[:, ci, qt * 128:(qt + 1) * 128],
                                wp_sb[:, ci, eo * 512:(eo + 1) * 512],
                                start=(ci == 0), stop=(ci == 1))
                    po_t = pout.tile([128, 2 * QT], F16, tag="po",
                                     name=f"po{qt}")
                    for eo in range(2):
                        oslc = po_t[:, eo * QT:(eo + 1) * QT]
                        eng = (cengs[eo] if cengs is not None
                               else nc.vector)
                        if eng is nc.scalar:
                            eng.copy(oslc, halves[eo])
                        else:
                            eng.tensor_copy(out=oslc, in_=halves[eo])
                        if dengs is not None:
                            # tail: drain each half as soon as it's copied,
                            # on its own queue
                            dengs[eo].dma_start(
                                y_d[qt * 128:(qt + 1) * 128,
                                    eo * QT:(eo + 1) * QT],
                                po_t[:, eo * QT:(eo + 1) * QT])
                    _oq[0] += 1
                    if dengs is None:
                        nc.gpsimd.dma_start(
                            y_d[qt * 128:(qt + 1) * 128, :], po_t[:])
                return emit

            # ---- attention pair pipeline ----
            _pend = [None]      # AV closure of the previous pair

            def s_pair(qj, h, pr):
                """Scores pair matmuls + exp + diagonal mask; returns the
                AV-emitter closure (run one pair later)."""
                hp = h // 2
                nkc = (qj + 1) * QT // KC
                ss = pss.tile([128, 2, QT], F32, tag="ss",
                              name=f"ss{qj}{h}{pr}")
                offs = [max(0, (2 * pr + j) * KC - qj * QT)
                        for j in range(2)]
                for j in range(2):
                    kc = 2 * pr + j
                    o = offs[j]
                    nc.tensor.matmul(
                        ss[:, j, o:QT],
                        kTpad[:, h, kc * KC:(kc + 1) * KC],
                        qT[:, hp, qj * QT + o:(qj + 1) * QT],
                        start=True, stop=True)
                pt = ptp.tile([128, 2, QT], F16, tag="pt",
                              name=f"pt{qj}{h}{pr}")
                if sum(offs) < 352:
                    # one exp for the pair; any dead-region garbage is
                    # never read (AV slices [o:])
                    nc.scalar.activation(
                        pt[:], ss[:],
                        mybir.ActivationFunctionType.Exp, scale=0.125)
                else:
                    # deep-diagonal pair: exp only live columns
                    for j, o in enumerate(offs):
                        nc.scalar.activation(
                            pt[:, j, o:QT], ss[:, j, o:QT],
                            mybir.ActivationFunctionType.Exp, scale=0.125)
                for j in range(2):
                    kc = 2 * pr + j
                    if kc * KC >= qj * QT:
                        # mask only the 128-wide diagonal band
                        o = offs[j]
                        w = min(KC, QT - o)
                        nc.gpsimd.affine_select(
                            out=pt[:, j, o:o + w],
                            in_=pt[:, j, o:o + w],
                            compare_op=mybir.AluOpType.is_ge,
                            fill=0.0, base=qj * QT + o - kc * KC,
                            pattern=[[1, w]],
                            channel_multiplier=-1)

                def av_emit(av):
                    for j in range(2):
                        kc = 2 * pr + j
                        o = offs[j]
                        nc.tensor.matmul(av[:, o:QT],
                                         vaug[:, kc, h, :],
                                         pt[:, j, o:QT],
                                         start=(kc == 0),
                                         stop=(kc == nkc - 1))
                    if pr == nkc // 2 - 1:  # last pair of this head
                        slab = (qj * HPC + h) % NSLAB
                        nc.vector.tensor_copy(out=yTun[:, slab, :],
                                              in_=av[:])
                return av_emit

            _avt = {}           # (qj, h) -> live av accumulator tile

            def attn_pair(qj, h, pr):
                av_emit = s_pair(qj, h, pr)
                if _pend[0] is not None:
                    _pend[0]()
                if pr == 0:
                    _avt[(qj, h)] = pav.tile([65, QT], F32, tag="av",
                                             name=f"av{qj}{h}")
                av = _avt[(qj, h)]
                _pend[0] = lambda: av_emit(av[:])

            def flush_av():
                if _pend[0] is not None:
                    _pend[0]()
                    _pend[0] = None

            # ---- normalization, split into two units so the bc matmul
            # never reaches the in-order PE queue before its reciprocal
            # (DMA -> Ln -> Exp -> DMA) is ready: a sub-us PE stall here
            # costs a multi-us HAM half-clock window.
            def recip_unit(qj, p):
                """Reciprocal of the rowsums for one head pair; rtp slot =
                pair index (the previous q-tile's reader is long done)."""
                def emit():
                    s0 = (qj * HPC + 2 * p) % NSLAB
                    rs2 = nrm.tile([2, QT], F32, tag="rs",
                                   name=f"rs{qj}{p}")
                    nc.sync.dma_start(rs2[:, :], yTun[64:65, s0:s0 + 2, :])
                    lg2 = nrm.tile([2, QT], F32, tag="lg",
                                   name=f"lg{qj}{p}")
                    nc.scalar.activation(lg2[:], rs2[:],
                                         mybir.ActivationFunctionType.Ln)
                    rt2 = nrm.tile([2, QT], F32R, tag="rt",
                                   name=f"rt{qj}{p}")
                    nc.scalar.activation(rt2[:], lg2[:],
                                         mybir.ActivationFunctionType.Exp,
                                         scale=-1.0)
                    for i in range(2):
                        nc.gpsimd.dma_start(rtp[32 * i:32 * i + 1, p, :],
                                            rt2[i:i + 1, :])
                return emit

            def recip4_unit(qj):
                """Batched reciprocal for all 4 heads of a q-tile: one
                gather, one Ln, one Exp (free size unchanged at 512)."""
                def emit():
                    s0 = (qj * HPC) % NSLAB
                    rs4 = nrm.tile([4, QT], F32, tag="rs",
                                   name=f"rs4{qj}")
                    nc.sync.dma_start(rs4[:, :], yTun[64:65, s0:s0 + 4, :])
                    lg4 = nrm.tile([4, QT], F32, tag="lg",
                                   name=f"lg4{qj}")
                    nc.scalar.activation(lg4[:], rs4[:],
                                         mybir.ActivationFunctionType.Ln)
                    rt4 = nrm.tile([4, QT], F32R, tag="rt",
                                   name=f"rt4{qj}")
                    nc.scalar.activation(rt4[:], lg4[:],
                                         mybir.ActivationFunctionType.Exp,
                                         scale=-1.0)
                    for h in range(HPC):
                        nc.gpsimd.dma_start(
                            rtp[32 * (h % 2):32 * (h % 2) + 1, h // 2, :],
                            rt4[h:h + 1, :])
                return emit

            def bcmul_unit(qj, p):
                def emit():
                    qslc = slice(qj * QT, (qj + 1) * QT)
                    bc = ps1.tile([128, QT], F32, tag="m",
                                  name=f"bc{qj}{p}")
                    nc.tensor.matmul(bc[:], ind33[:, :], rtp[:, p, :],
                                     start=True, stop=True)
                    for i in range(2):
                        h = 2 * p + i
                        slab = (qj * HPC + h) % NSLAB
                        po = 64 * i
                        with nc.allow_low_precision(reason="proj lhsT"):
                            nc.vector.tensor_tensor(
                                out=yT[po:po + 64, p, qslc],
                                in0=yTun[0:64, slab, :],
                                in1=bc[po:po + 64, :],
                                op=mybir.AluOpType.mult)
                return emit

            # ---- merged pipeline ----
            qk_ec_major()
            v_unit(0)()
            v_unit(1)()

            # filler schedule per q-tile: {slot: [units]} -- slots are
            # attention-pair indices; units placed just before their
            # consumers need them so the PE stream never runs dry (HAM
            # keeps the PE clock at 2.4 GHz only while it stays busy).
            fills = {
                0: {0: [v_unit(2)], 1: [v_unit(3)], 2: [v_unit(4)],
                    3: [v_unit(5)], 4: [v_unit(6)], 5: [v_unit(7)],
                    6: [qk23_unit(2, 2)], 7: [qk23_unit(3, 2)]},
                1: {0: [recip4_unit(0)], 1: [v_unit(8)],
                    2: [v_unit(9)],
                    4: [bcmul_unit(0, 0)], 5: [bcmul_unit(0, 1)],
                    7: [v_unit(10)],
                    9: [v_unit(11)], 11: [qk23_unit(0, 2)],
                    13: [qk23_unit(1, 2)]},
                2: {0: [recip4_unit(1)], 1: [proj_unit(0)],
                    2: [qk23_unit(0, 3)], 3: [proj_unit(1)],
                    4: [bcmul_unit(1, 0)], 5: [bcmul_unit(1, 1)],
                    6: [qk23_unit(1, 3)], 7: [proj_unit(2)],
                    9: [proj_unit(3)], 11: [proj_unit(4)],
                    13: [proj_unit(5)], 15: [proj_unit(6)],
                    17: [proj_unit(7)],
                    18: [v_unit(12)], 19: [v_unit(13)],
                    20: [v_unit(14)], 21: [v_unit(15)]},
                3: {0: [recip4_unit(2)], 1: [qk23_unit(2, 3)],
                    2: [qk23_unit(3, 3)], 4: [bcmul_unit(2, 0)],
                    5: [bcmul_unit(2, 1)],
                    8: [proj_unit(8)],
                    13: [proj_unit(9)], 18: [recip_unit(3, 0)],
                    24: [bcmul_unit(3, 0)],
                    26: [proj_unit(10, dengs=(nc.sync, nc.scalar))]},
            }

            for qj in range(4):
                npairs = (qj + 1) * QT // KC // 2
                idx = 0
                for h in range(HPC):
                    for pr in range(npairs):
                        attn_pair(qj, h, pr)
                        for u in fills[qj].get(idx, ()):
                            u()
                        idx += 1
            # tail: the last recip chain runs under proj 10/11 matmuls so
            # the PE never idles while it resolves
            flush_av()
            recip_unit(3, 1)()
            proj_unit(11, dengs=(nc.scalar, nc.sync))()
            bcmul_unit(3, 1)()
            proj_unit(12, cengs=(nc.scalar, nc.vector),
                      dengs=(nc.sync, nc.scalar))()
            proj_unit(13, cengs=(nc.vector, nc.scalar),
                      dengs=(nc.scalar, nc.sync))()
            proj_unit(14, cengs=(nc.scalar, nc.vector),
                      dengs=(nc.sync, nc.scalar))()
            proj_unit(15, cengs=(nc.vector, nc.scalar),
                      dengs=(nc.scalar, nc.sync))()
    return nc


# ---------------------------------------------------------------- host

_NC_CACHE = []


def _get_nc():
    if not _NC_CACHE:
        _install_bir_fix()
        _NC_CACHE.append(build())
    return _NC_CACHE[0]


def make_in_maps(x, w_attn, w_proj):
    in_maps = []
    for c in range(N_CORES):
        b, h0 = c // 4, (c % 4) * HPC
        wq = w_attn[:, h0 * D:(h0 + HPC) * D]
        wk = w_attn[:, E + h0 * D:E + (h0 + HPC) * D]
        wv = w_attn[:, 2 * E + h0 * D:2 * E + (h0 + HPC) * D]
        def pmaj(a, dt):
            # [n*128, c] row-chunked -> partition-major [128, n, c]
            n = a.shape[0] // 128
            return np.ascontiguousarray(
                a.reshape(n, 128, -1).transpose(1, 0, 2).astype(dt))

        in_maps.append({
            "xT": pmaj(x[b].T, np.float16),
            "wqk": pmaj(np.concatenate([wq, wk], axis=1), np.float16),
            "wv": pmaj(wv, np.float16),
            "wproj": pmaj(w_proj[h0 * D:(h0 + HPC) * D, :], np.float16),
        })
    return in_maps


def run(x, w_attn, w_proj, trace=False, tmpdir=None):
    from concourse.bass_utils import run_bass_kernel_spmd
    nc = _get_nc()
    res = run_bass_kernel_spmd(nc, make_in_maps(x, w_attn, w_proj),
                               list(range(N_CORES)), trace=trace, tmpdir=tmpdir)
    y = np.zeros((B, S, E), np.float32)
    for c in range(N_CORES):
        y[c // 4] += res.results[c]["y"].astype(np.float32)
    return y, res


def kernel(x, w_attn, w_proj):
    y, _ = run(np.asarray(x, np.float32), np.asarray(w_attn, np.float32),
               np.asarray(w_proj, np.float32))
    return y
